# revision 26
# baseline (speedup 1.0000x reference)
"""Trainium2 Bass kernel for elementwise i1e(z) = exp(-|z|) * I1(z), f32.

Input z: [64, 1024, 1024] f32 with |z| <= 5.42 (randn). Sharded across 8
NeuronCores along the leading dim (8 slabs of [8, 1024, 1024]).

Strategy (memory-regime): ship z as fp16 (halves HBM traffic vs f32; the
2e-2 tolerance leaves ample room), compute on device as

    i1e(x) = x * exp(CK*((|x|+CB)^2 + CC)*|x| + CD)

(minimax cubic-exp fit, graded rel err ~4.2e-3) with the whole cubic done
by ONE custom fused DVE op (single 1x pass over the data):

    m = (sq(|x|+CB)+CC)*|x|    DVE custom IVE_CUBIC_ANT  (~4.45us/full tile)
    E = exp(CK*m + CD)         ScalarE Exp, in place     (~3.7us)
    o = x*E                    DVE TT fp16 2x (~2.3us) or GPSIMD TT
                               (6 middle tiles, split in halves, ~4.1us/half)

vs the stock-op pipeline this frees ScalarE from the Square pass and
collapses abs/add/square/add into one DVE instruction, cutting DVE busy
from ~119us to ~96us/core. Six o-multiplies go to GPSIMD (Q7, 0.42 eff)
to keep DVE near the fp16 DMA floor.

Measured on trn2 (8 cores, SPMD): ~124-130us/core vs ~137us baseline.
DMA-only floor for the same 32MB/core is ~119us with this issue scheme and
the steady-state pipeline period is ~115us (For_i repeat-slope), so the
kernel runs within ~10% of its I/O-pipeline bound. Loads are issued from
the ACT queue (their pin-recycle waits are satisfied early; on the SP
queue they head-of-line block behind stores waiting on compute), stores
from SP.
"""
import sys

sys.path.insert(0, "/opt/trn_rl_repo")

import numpy as np

import concourse.bacc as bacc
import concourse.bass as bass
import concourse.mybir as mybir
import concourse.tile as tile
from concourse.bass_utils import run_bass_kernel_spmd

import concourse.dve_ops as dve_ops
from concourse.dve_spec import Spec, Src0, Zero, C0, C1, sq, maxx, lower
from concourse.dve_spec import _has_src1
from concourse.dve_uop import DveOpSpec

N_CORES = 8
P = 128
F = 4096
TILES = 16  # per-core elems: 8*1024*1024 = TILES * P * F

# cubic-exp fit params (end-to-end fp16 minimax on y in [0, 5.6])
# arg = CK*((y+CB)^2 + CC)*y + CD,  y = |x|
CB = -8.456950205885828
CC = 60.96620543563405
CK = -0.007439406983278787
CD = -0.7019528653977755

# full-tile items whose o=x*E multiply runs on GPSIMD instead of DVE
N_GPSIMD_O = 6

_cache = {}


def _fused_cubic_op():
    """Register IVE_CUBIC_ANT: out = (sq(|in0|+s0)+s1)*|in0| as a custom DVE
    op (single 1x pass; abs via maxx(x, 0-x); 6 ALU stages)."""
    name = "IVE_CUBIC_ANT"
    if any(o.name == name for o in dve_ops.OPS):
        return next(o for o in dve_ops.OPS if o.name == name)
    y = maxx(Src0, Zero - Src0)
    spec = Spec(
        body=(sq(y + C0) + C1) * y,
        reference=lambda in0, s0, s1: (((np.abs(in0) + s0) ** 2) + s1)
        * np.abs(in0),
    )
    opcode = dve_ops._CUSTOM_DVE_ROW_BASE + len(dve_ops.OPS)
    uops = lower(spec, ver="v3")
    sha = DveOpSpec(name=name, opcode=opcode, uops=uops, rd1_en=_has_src1(spec)).sha(
        "v3"
    )
    op = dve_ops.DveOp(name, spec, subdim=False, uops_sha={"v3": sha})
    dve_ops.OPS.append(op)
    dve_ops._SUB_OPCODE_FOR_NAME[name] = opcode
    dve_ops.CUSTOM_DVE_SPECS[name] = spec
    return op


def _register_const(nc, value):
    t = nc.alloc_sbuf_tensor(f"const-f32-{value}", [128, 1], mybir.dt.float32)
    nc.gpsimd.memset(t.ap(), value)
    nc.const_aps.aps[(mybir.dt.float32, value)] = t.ap()


def _build(repeat: int = 0, dma_only: bool = False, n_gpsimd: int = N_GPSIMD_O):
    fused = _fused_cubic_op()
    nc = bacc.Bacc("TRN2", target_bir_lowering=False, debug=False)
    dt = mybir.dt.float16
    _register_const(nc, CD)
    nc.all_engine_barrier()
    x_d = nc.dram_tensor("z_in", [TILES * P, F], dt, kind="ExternalInput")
    o_d = nc.dram_tensor("out", [TILES * P, F], dt, kind="ExternalOutput")
    EXP = mybir.ActivationFunctionType.Exp
    OP = mybir.AluOpType

    def body(pin, pout, ptmp):
        # Work items: (row0, col0, flen, use_gpsimd). First/last tiles are
        # split into quarters so the pipeline fills and drains quickly.
        items = []
        for it in range(TILES):
            if it == 0 or it == TILES - 1:
                for j in range(4):
                    items.append([it * P, j * (F // 4), F // 4, False])
            else:
                items.append([it * P, 0, F, False])
        # spread GPSIMD-offloaded o-mults over middle full tiles
        full_idx = [i for i, w in enumerate(items) if w[2] == F]
        for j in range(min(n_gpsimd, len(full_idx))):
            items[full_idx[(j * len(full_idx)) // max(n_gpsimd, 1)]][3] = True

        live = {}

        def st_load(i):
            r, c, fl, _ = items[i]
            xs = pin.tile([P, F], dt, tag="x")
            # issue loads from the ACT queue: its waits (pin buffer reuse)
            # are satisfied far in advance, so loads never sit blocked behind
            # a store the way they do on the shared SP queue.
            nc.scalar.dma_start(xs[:, :fl], x_d[r : r + P, c : c + fl])
            if dma_only:
                nc.sync.dma_start(o_d[r : r + P, c : c + fl], xs[:, :fl])
                return
            live[i] = {"x": xs}

        def st_m(i):
            r, c, fl, _ = items[i]
            d = live[i]
            m = ptmp.tile([P, F], dt, tag="m")
            nc.vector._custom_dve(
                fused, out=m[:, :fl], in0=d["x"][:, :fl], s0=CB, s1=CC
            )
            d["m"] = m

        def st_E(i):
            r, c, fl, _ = items[i]
            m = live[i]["m"]
            nc.scalar.activation(m[:, :fl], m[:, :fl], EXP, bias=CD, scale=CK)

        def st_o(i):
            r, c, fl, use_g = items[i]
            d = live.pop(i)
            o = pout.tile([P, F], dt, tag="o")
            if use_g:
                # split the slow GPSIMD multiply in half so stores start
                # earlier and x/m buffers free sooner (finer splits drown in
                # Q7 per-instruction overhead)
                h = fl // 2
                for c0 in range(0, fl, h):
                    nc.gpsimd.tensor_tensor(
                        o[:, c0 : c0 + h], d["x"][:, c0 : c0 + h],
                        d["m"][:, c0 : c0 + h], OP.mult,
                    )
                    nc.sync.dma_start(
                        o_d[r : r + P, c + c0 : c + c0 + h], o[:, c0 : c0 + h]
                    )
            elif fl == F:
                # halve the DVE multiply too: the first half's store departs
                # ~1.2us earlier and buffers recycle sooner
                h = fl // 2
                for c0 in range(0, fl, h):
                    nc.vector.tensor_tensor(
                        o[:, c0 : c0 + h], d["x"][:, c0 : c0 + h],
                        d["m"][:, c0 : c0 + h], OP.mult,
                    )
                    nc.sync.dma_start(
                        o_d[r : r + P, c + c0 : c + c0 + h], o[:, c0 : c0 + h]
                    )
            else:
                nc.vector.tensor_tensor(
                    o[:, :fl], d["x"][:, :fl], d["m"][:, :fl], OP.mult
                )
                nc.sync.dma_start(o_d[r : r + P, c : c + fl], o[:, :fl])

        stages = [st_load, st_m, st_E, st_o]
        STAGES = len(stages)
        n_items = len(items)
        for step in range(n_items + STAGES - 1):
            for k in reversed(range(STAGES)):
                i = step - k
                if 0 <= i < n_items and not (dma_only and k > 0):
                    stages[k](i)

    with tile.TileContext(nc) as tc:
        with (
            tc.tile_pool(name="io_in", bufs=11) as pin,
            tc.tile_pool(name="io_out", bufs=6) as pout,
            tc.tile_pool(name="tmp", bufs=7) as ptmp,
        ):
            if repeat:
                with tc.For_i(0, repeat, 1, staggered_reset=True):
                    body(pin, pout, ptmp)
            else:
                body(pin, pout, ptmp)
    nc.finalize()
    return nc


def kernel(z: np.ndarray) -> np.ndarray:
    if "nc" not in _cache:
        _cache["nc"] = _build()
    nc = _cache["nc"]
    z16 = np.asarray(z, dtype=np.float16)
    rows = z16.shape[0] // N_CORES  # 8
    in_maps = [
        {"z_in": z16[i * rows : (i + 1) * rows].reshape(TILES * P, F)}
        for i in range(N_CORES)
    ]
    res = run_bass_kernel_spmd(nc, in_maps, list(range(N_CORES)))
    _cache["last_results"] = res
    out = np.concatenate(
        [
            res.results[i]["out"]
            .reshape(rows, z.shape[1], z.shape[2])
            .astype(np.float32)
            for i in range(N_CORES)
        ],
        axis=0,
    )
    return out


# revision 27
# speedup vs baseline: 1.0795x; 1.0795x over previous
"""Trainium2 Bass kernel for elementwise i1e(z) = exp(-|z|) * I1(z), f32.

Input z: [64, 1024, 1024] f32 with |z| <= 5.42 (randn). Sharded across 8
NeuronCores along the leading dim (8 slabs of [8, 1024, 1024]).

Strategy (memory-regime): ship z as fp16 (halves HBM traffic vs f32; the
2e-2 tolerance leaves ample room), compute on device as

    i1e(x) = x * exp(CK*((|x|+CB)^2 + CC)*|x| + CD)

(minimax cubic-exp fit, graded rel err ~4.2e-3) with the whole cubic done
by ONE custom fused DVE op (single 1x pass over the data):

    m = (sq(|x|+CB)+CC)*|x|    DVE custom IVE_CUBIC_ANT  (~4.45us/full tile)
    E = exp(CK*m + CD)         ScalarE Exp, in place     (~3.7us)
    o = x*E                    DVE TT fp16 2x (~2.3us) or GPSIMD TT
                               (6 middle tiles, split in halves, ~4.1us/half)

vs the stock-op pipeline this frees ScalarE from the Square pass and
collapses abs/add/square/add into one DVE instruction, cutting DVE busy
from ~119us to ~96us/core. Six o-multiplies go to GPSIMD (Q7, 0.42 eff)
to keep DVE near the fp16 DMA floor.

Measured on trn2 (8 cores, SPMD): ~124-130us/core vs ~137us baseline.
DMA-only floor for the same 32MB/core is ~119us with this issue scheme and
the steady-state pipeline period is ~115us (For_i repeat-slope), so the
kernel runs within ~10% of its I/O-pipeline bound. Loads are issued from
the ACT queue (their pin-recycle waits are satisfied early; on the SP
queue they head-of-line block behind stores waiting on compute), stores
from SP.
"""
import sys

sys.path.insert(0, "/opt/trn_rl_repo")

import numpy as np

import concourse.bacc as bacc
import concourse.bass as bass
import concourse.mybir as mybir
import concourse.tile as tile
from concourse.bass_utils import run_bass_kernel_spmd

import concourse.dve_ops as dve_ops
from concourse.dve_spec import Spec, Src0, Zero, C0, C1, sq, maxx, lower
from concourse.dve_spec import _has_src1
from concourse.dve_uop import DveOpSpec

N_CORES = 8
P = 128
F = 4096
TILES = 16  # per-core elems: 8*1024*1024 = TILES * P * F

# cubic-exp fit params (end-to-end fp16 minimax on y in [0, 5.6])
# arg = CK*((y+CB)^2 + CC)*y + CD,  y = |x|
CB = -8.456950205885828
CC = 60.96620543563405
CK = -0.007439406983278787
CD = -0.7019528653977755

# full-tile items whose o=x*E multiply runs on GPSIMD instead of DVE
N_GPSIMD_O = 6

_cache = {}


def _fused_cubic_op():
    """Register IVE_CUBIC_ANT: out = (sq(|in0|+s0)+s1)*|in0| as a custom DVE
    op (single 1x pass; abs via maxx(x, 0-x); 6 ALU stages)."""
    name = "IVE_CUBIC_ANT"
    if any(o.name == name for o in dve_ops.OPS):
        return next(o for o in dve_ops.OPS if o.name == name)
    y = maxx(Src0, Zero - Src0)
    spec = Spec(
        body=(sq(y + C0) + C1) * y,
        reference=lambda in0, s0, s1: (((np.abs(in0) + s0) ** 2) + s1)
        * np.abs(in0),
    )
    opcode = dve_ops._CUSTOM_DVE_ROW_BASE + len(dve_ops.OPS)
    uops = lower(spec, ver="v3")
    sha = DveOpSpec(name=name, opcode=opcode, uops=uops, rd1_en=_has_src1(spec)).sha(
        "v3"
    )
    op = dve_ops.DveOp(name, spec, subdim=False, uops_sha={"v3": sha})
    dve_ops.OPS.append(op)
    dve_ops._SUB_OPCODE_FOR_NAME[name] = opcode
    dve_ops.CUSTOM_DVE_SPECS[name] = spec
    return op


def _register_const(nc, value):
    t = nc.alloc_sbuf_tensor(f"const-f32-{value}", [128, 1], mybir.dt.float32)
    nc.gpsimd.memset(t.ap(), value)
    nc.const_aps.aps[(mybir.dt.float32, value)] = t.ap()


def _build(repeat: int = 0, dma_only: bool = False, n_gpsimd: int = N_GPSIMD_O):
    fused = _fused_cubic_op()
    nc = bacc.Bacc("TRN2", target_bir_lowering=False, debug=False)
    dt = mybir.dt.float16
    _register_const(nc, CD)
    nc.all_engine_barrier()
    x_d = nc.dram_tensor("z_in", [TILES * P, F], dt, kind="ExternalInput")
    o_d = nc.dram_tensor("out", [TILES * P, F], dt, kind="ExternalOutput")
    EXP = mybir.ActivationFunctionType.Exp
    OP = mybir.AluOpType

    def body(pin, pout, ptmp):
        # Work items: (row0, col0, flen, use_gpsimd). First/last tiles are
        # split into quarters so the pipeline fills and drains quickly.
        items = []
        for it in range(TILES):
            if it == 0 or it == TILES - 1:
                for j in range(4):
                    items.append([it * P, j * (F // 4), F // 4, False])
            else:
                items.append([it * P, 0, F, False])
        # spread GPSIMD-offloaded o-mults over middle full tiles
        full_idx = [i for i, w in enumerate(items) if w[2] == F]
        for j in range(min(n_gpsimd, len(full_idx))):
            items[full_idx[(j * len(full_idx)) // max(n_gpsimd, 1)]][3] = True

        live = {}

        def st_load(i):
            r, c, fl, _ = items[i]
            xs = pin.tile([P, F], dt, tag="x")
            # issue loads from the ACT queue: its waits (pin buffer reuse)
            # are satisfied far in advance, so loads never sit blocked behind
            # a store the way they do on the shared SP queue.
            nc.scalar.dma_start(xs[:, :fl], x_d[r : r + P, c : c + fl])
            if dma_only:
                nc.sync.dma_start(o_d[r : r + P, c : c + fl], xs[:, :fl])
                return
            live[i] = {"x": xs}

        def st_m(i):
            r, c, fl, _ = items[i]
            d = live[i]
            m = ptmp.tile([P, F], dt, tag="m")
            nc.vector._custom_dve(
                fused, out=m[:, :fl], in0=d["x"][:, :fl], s0=CB, s1=CC
            )
            d["m"] = m

        def st_E(i):
            r, c, fl, _ = items[i]
            m = live[i]["m"]
            nc.scalar.activation(m[:, :fl], m[:, :fl], EXP, bias=CD, scale=CK)

        def st_o(i):
            r, c, fl, use_g = items[i]
            d = live.pop(i)
            o = pout.tile([P, F], dt, tag="o")
            if use_g:
                # split the slow GPSIMD multiply in half so stores start
                # earlier and x/m buffers free sooner (finer splits drown in
                # Q7 per-instruction overhead)
                h = fl // 2
                for c0 in range(0, fl, h):
                    nc.gpsimd.tensor_tensor(
                        o[:, c0 : c0 + h], d["x"][:, c0 : c0 + h],
                        d["m"][:, c0 : c0 + h], OP.mult,
                    )
                    nc.sync.dma_start(
                        o_d[r : r + P, c + c0 : c + c0 + h], o[:, c0 : c0 + h]
                    )
            else:
                nc.vector.tensor_tensor(
                    o[:, :fl], d["x"][:, :fl], d["m"][:, :fl], OP.mult
                )
                nc.sync.dma_start(o_d[r : r + P, c : c + fl], o[:, :fl])

        stages = [st_load, st_m, st_E, st_o]
        STAGES = len(stages)
        n_items = len(items)
        for step in range(n_items + STAGES - 1):
            for k in reversed(range(STAGES)):
                i = step - k
                if 0 <= i < n_items and not (dma_only and k > 0):
                    stages[k](i)

    with tile.TileContext(nc) as tc:
        with (
            tc.tile_pool(name="io_in", bufs=11) as pin,
            tc.tile_pool(name="io_out", bufs=6) as pout,
            tc.tile_pool(name="tmp", bufs=7) as ptmp,
        ):
            if repeat:
                with tc.For_i(0, repeat, 1, staggered_reset=True):
                    body(pin, pout, ptmp)
            else:
                body(pin, pout, ptmp)
    nc.finalize()
    return nc


def kernel(z: np.ndarray) -> np.ndarray:
    if "nc" not in _cache:
        _cache["nc"] = _build()
    nc = _cache["nc"]
    z16 = np.asarray(z, dtype=np.float16)
    rows = z16.shape[0] // N_CORES  # 8
    in_maps = [
        {"z_in": z16[i * rows : (i + 1) * rows].reshape(TILES * P, F)}
        for i in range(N_CORES)
    ]
    res = run_bass_kernel_spmd(nc, in_maps, list(range(N_CORES)))
    _cache["last_results"] = res
    out = np.concatenate(
        [
            res.results[i]["out"]
            .reshape(rows, z.shape[1], z.shape[2])
            .astype(np.float32)
            for i in range(N_CORES)
        ],
        axis=0,
    )
    return out
